# revision 17
# baseline (speedup 1.0000x reference)
"""Trainium2 Bass kernel for BatchRemoveQuatDiscontinuities.

Algorithm (per (batch, joint) lane):
    d[t]    = dot(q[t], q[t-1])                (fp32, 4-wide dot)
    flip[t] = 1 if d[t] < 0 else 0             (t >= 1; flip[0] = 0)
    sigma[t] = (-1)^(sum_{s<=t} flip[s])       (cumulative sign parity)
    out[t]  = q[t] * sigma[t]

Mapping on a NeuronCore (data-parallel over batch across 8 cores):
  * Tiles put time on the partition axis: tile i holds q rows
    t in [127*i - 1, 127*i + 126] (one-row overlap), free dim = (b, j, c).
  * The one-step time shift is a TensorE matmul with an off-diagonal
    0/1 matrix S (fp32: the single 1.0*q product is exact).
  * prod = q .* q_shift on VectorE, dot via tensor_reduce over c.
  * flip indicator on ScalarE: e = Relu(Sign(-d)) in bf16.
  * The prefix sum over time (partitions) is a TensorE matmul with an
    upper-triangular ones matrix (bf16 inputs, fp32 PSUM accumulate:
    counts <= 1024 stay exact).  e[0] is 0 by construction, so the
    cross-tile carry (prefix parity, 0/1, bf16-exact) is DMA'd into
    e[0] of the next tile; the triangular matmul then adds it to every
    row's count, which is all the parity needs.
  * Parity: VectorE casts the prefix to int32 and takes (n & 1);
    ScalarE turns it into sigma = 1 - 2*(n & 1) in {+1, -1} (bf16).
    GpSimd multiplies out = q * sigma (broadcast over c) — a +/-1.0
    multiply, exact in fp32.
"""

import numpy as np
import ml_dtypes
from contextlib import ExitStack

import concourse.bass as bass
import concourse.bacc as bacc
import concourse.tile as tile
from concourse import mybir
from concourse.bass_utils import run_bass_kernel_spmd

B, T, J, C = 128, 1024, 64, 4
NCORES = 8
JC = J * C                      # 256 floats, 1KB contiguous per (b, t)
BPC = B // NCORES               # 16 batch clips per core
BCHUNK = 8                      # clips per tile (free dim = BCHUNK*JC = 2048)
TSTRIDE = 127                   # valid t rows produced per full tile
MMN = 512                       # max matmul free dim (one PSUM bank fp32)

FP32 = mybir.dt.float32
BF16 = mybir.dt.bfloat16
I32 = mybir.dt.int32
Alu = mybir.AluOpType
Act = mybir.ActivationFunctionType


def _bcast_c(ap3, c=C):
    """[P, S] AP -> [P, S, c] AP broadcasting along a new innermost dim."""
    return bass.AP(
        tensor=ap3.tensor,
        offset=ap3.offset,
        ap=[*[list(d) for d in ap3.ap], [0, c]],
    )


def build_nc(bpc=BPC, t=T, bchunk=BCHUNK):
    nc = bacc.Bacc(None, target_bir_lowering=False)
    q = nc.declare_dram_parameter("q", [bpc, t, J, C], FP32, isOutput=False)
    smat = nc.declare_dram_parameter("smat", [128, 128], FP32, isOutput=False)
    lmat = nc.declare_dram_parameter("lmat", [128, 128], BF16, isOutput=False)
    out = nc.declare_dram_parameter("out", [bpc, t, J, C], FP32, isOutput=True)

    nchunks = bpc // bchunk
    fd = bchunk * JC            # tile free dim (2048)
    hfd = fd // 2               # half free dim (1024)
    sd = bchunk * J             # dot free dim (512)
    ntiles = (t + TSTRIDE - 1) // TSTRIDE

    with tile.TileContext(nc) as tc, ExitStack() as ctx:
        consts = ctx.enter_context(tc.tile_pool(name="consts", bufs=1))
        qpool = ctx.enter_context(tc.tile_pool(name="qpool", bufs=3))
        opool = ctx.enter_context(tc.tile_pool(name="opool", bufs=3))
        spool = ctx.enter_context(tc.tile_pool(name="spool", bufs=3))
        qpvp = ctx.enter_context(tc.tile_pool(name="qpvp", bufs=3, space="PSUM"))
        prefp = ctx.enter_context(tc.tile_pool(name="prefp", bufs=2, space="PSUM"))

        smatSB = consts.tile([128, 128], FP32)
        nc.sync.dma_start(out=smatSB[:, :], in_=smat[:, :])
        lmatSB = consts.tile([128, 128], BF16)
        nc.sync.dma_start(out=lmatSB[:, :], in_=lmat[:, :])

        for cc in range(nchunks):
            b0 = cc * bchunk
            sig_prev = None
            for i in range(ntiles):
                t0 = TSTRIDE * i
                nv = min(TSTRIDE, t - t0)
                rows = nv + 1

                qt = qpool.tile([128, fd], FP32, tag="qt")
                qt3 = qt.rearrange("p (b x) -> p b x", b=bchunk)
                if i == 0:
                    nc.sync.dma_start(
                        out=qt3[1:rows],
                        in_=q[b0:b0 + bchunk, 0:nv, :, :]
                        .rearrange("b t j c -> t b (j c)"),
                    )
                    nc.sync.dma_start(
                        out=qt3[0:1],
                        in_=q[b0:b0 + bchunk, 0:1, :, :]
                        .rearrange("b t j c -> t b (j c)"),
                    )
                else:
                    nc.sync.dma_start(
                        out=qt3[0:rows],
                        in_=q[b0:b0 + bchunk, t0 - 1:t0 + nv, :, :]
                        .rearrange("b t j c -> t b (j c)"),
                    )

                # o holds prod, later overwritten with the final output
                o = opool.tile([128, fd], FP32, tag="o")
                for h in range(2):
                    qpv = qpvp.tile([128, hfd], FP32, tag="qpv")
                    for lo in range(0, hfd, MMN):
                        w = min(MMN, hfd - lo)
                        nc.tensor.matmul(
                            qpv[:rows, lo:lo + w],
                            lhsT=smatSB[:rows, :rows],
                            rhs=qt[:rows, h * hfd + lo:h * hfd + lo + w],
                            start=True,
                            stop=True,
                        )
                    nc.vector.tensor_tensor(
                        out=o[:rows, h * hfd:(h + 1) * hfd],
                        in0=qt[:rows, h * hfd:(h + 1) * hfd],
                        in1=qpv[:rows],
                        op=Alu.mult,
                    )

                # d = pairwise dot over c:  (c0+c1) + (c2+c3)
                u = spool.tile([128, 2 * sd], FP32, tag="u")
                ov = o[:rows].rearrange("p (s k two) -> p s k two", k=2, two=2)
                uv = u[:rows].rearrange("p (s two) -> p s two", two=2)
                nc.vector.tensor_tensor(
                    out=uv, in0=ov[:, :, :, 0], in1=ov[:, :, :, 1], op=Alu.add
                )
                dt_ = spool.tile([128, sd], FP32, tag="dt")
                nc.vector.tensor_tensor(
                    out=dt_[:rows], in0=uv[:, :, 0], in1=uv[:, :, 1], op=Alu.add
                )

                sg = spool.tile([128, sd], FP32, tag="sg")
                nc.scalar.activation(sg[:rows], dt_[:rows], Act.Sign, scale=-1.0)
                e = spool.tile([128, sd], BF16, tag="e")
                nc.scalar.activation(e[:rows], sg[:rows], Act.Relu)
                if i == 0:
                    nc.scalar.mul(e[0:1], e[0:1], 0.0)
                else:
                    # carry: previous tile's sigma[last] -> parity into e[0]
                    sigscr = spool.tile([1, sd], BF16, tag="sigscr")
                    nc.sync.dma_start(out=sigscr[0:1], in_=sig_prev)
                    nc.scalar.activation(
                        e[0:1], sigscr[0:1], Act.Copy, bias=0.5, scale=-0.5
                    )

                pref = prefp.tile([128, sd], FP32, tag="pref")
                nc.tensor.matmul(
                    pref[:rows],
                    lhsT=lmatSB[:rows, :rows],
                    rhs=e[:rows],
                    start=True,
                    stop=True,
                )

                # parity -> sigma in {+1, -1}: int cast, &1, then 1 - 2*p
                prefi = spool.tile([128, sd], I32, tag="prefi")
                nc.vector.tensor_copy(out=prefi[:rows], in_=pref[:rows])
                p01 = spool.tile([128, sd], I32, tag="p01")
                nc.vector.tensor_scalar(
                    out=p01[:rows], in0=prefi[:rows], scalar1=1, scalar2=None,
                    op0=Alu.bitwise_and,
                )
                sig = spool.tile([128, sd], BF16, tag="sig")
                nc.scalar.activation(
                    sig[:rows], p01[:rows], Act.Copy, bias=1.0, scale=-2.0
                )
                sig_prev = sig[rows - 1:rows]

                # out = q * sigma (broadcast over c) -> multiply by +/-1
                nc.gpsimd.tensor_tensor(
                    out=o[:rows].rearrange("p (s c) -> p s c", c=C),
                    in0=qt[:rows].rearrange("p (s c) -> p s c", c=C),
                    in1=_bcast_c(sig[:rows]),
                    op=Alu.mult,
                )

                nc.sync.dma_start(
                    out=out[b0:b0 + bchunk, t0:t0 + nv, :, :]
                    .rearrange("b t j c -> t b (j c)"),
                    in_=o.rearrange("p (b x) -> p b x", b=bchunk)[1:1 + nv],
                )
    return nc


def make_consts():
    smat = np.eye(128, k=1, dtype=np.float32)      # S[k, m] = 1 iff m == k+1
    lmat = np.triu(np.ones((128, 128), np.float32)).astype(ml_dtypes.bfloat16)
    return smat, lmat


def kernel(joint_rotations: np.ndarray) -> np.ndarray:
    q = np.ascontiguousarray(joint_rotations, dtype=np.float32)
    assert q.shape == (B, T, J, C)
    smat, lmat = make_consts()
    nc = build_nc()
    nc.finalize()   # run bacc passes (wait splitting, reg alloc) + freeze
    in_maps = [
        {"q": q[c * BPC:(c + 1) * BPC], "smat": smat, "lmat": lmat}
        for c in range(NCORES)
    ]
    res = run_bass_kernel_spmd(nc, in_maps, list(range(NCORES)))
    outs = [np.asarray(r["out"]) for r in res.results]
    return np.concatenate(outs, axis=0)


# revision 19
# speedup vs baseline: 11.9298x; 11.9298x over previous
"""Trainium2 Bass kernel for BatchRemoveQuatDiscontinuities.

Algorithm (per (batch, joint) lane):
    d[t]    = dot(q[t], q[t-1])                (fp32, 4-wide dot)
    flip[t] = 1 if d[t] < 0 else 0             (t >= 1; flip[0] = 0)
    sigma[t] = (-1)^(sum_{s<=t} flip[s])       (cumulative sign parity)
    out[t]  = q[t] * sigma[t]

Mapping on a NeuronCore (data-parallel over batch across 8 cores):
  * Tiles put time on the partition axis: tile i holds q rows
    t in [127*i - 1, 127*i + 126] (one-row overlap), free dim = (b, j, c).
  * The one-step time shift is a TensorE matmul with an off-diagonal
    0/1 matrix S (fp32: the single 1.0*q product is exact).
  * prod = q .* q_shift on VectorE, dot via tensor_reduce over c.
  * flip indicator on ScalarE: e = Relu(Sign(-d)) in bf16.
  * The prefix sum over time (partitions) is a TensorE matmul with an
    upper-triangular ones matrix (bf16 inputs, fp32 PSUM accumulate:
    counts <= 1024 stay exact).  e[0] is 0 by construction, so the
    cross-tile carry (prefix parity, 0/1, bf16-exact) is DMA'd into
    e[0] of the next tile; the triangular matmul then adds it to every
    row's count, which is all the parity needs.
  * Parity: VectorE casts the prefix to int32 and takes (n & 1);
    ScalarE turns it into sigma = 1 - 2*(n & 1) in {+1, -1} (bf16).
    GpSimd multiplies out = q * sigma (broadcast over c) — a +/-1.0
    multiply, exact in fp32.
"""

import numpy as np
import ml_dtypes
from contextlib import ExitStack

import concourse.bass as bass
import concourse.bacc as bacc
import concourse.tile as tile
from concourse import mybir
from concourse.bass_utils import run_bass_kernel_spmd

B, T, J, C = 128, 1024, 64, 4
NCORES = 8
JC = J * C                      # 256 floats, 1KB contiguous per (b, t)
BPC = B // NCORES               # 16 batch clips per core
BCHUNK = 8                      # clips per tile (free dim = BCHUNK*JC = 2048)
TSTRIDE = 127                   # valid t rows produced per full tile
MMN = 512                       # max matmul free dim (one PSUM bank fp32)

FP32 = mybir.dt.float32
BF16 = mybir.dt.bfloat16
I32 = mybir.dt.int32
Alu = mybir.AluOpType
Act = mybir.ActivationFunctionType


def _bcast_c(ap3, c=C):
    """[P, S] AP -> [P, S, c] AP broadcasting along a new innermost dim."""
    return bass.AP(
        tensor=ap3.tensor,
        offset=ap3.offset,
        ap=[*[list(d) for d in ap3.ap], [0, c]],
    )


def build_nc(bpc=BPC, t=T, bchunk=BCHUNK, reps=1):
    nc = bacc.Bacc(None, target_bir_lowering=False)
    q = nc.declare_dram_parameter("q", [bpc, t, J, C], FP32, isOutput=False)
    smat = nc.declare_dram_parameter("smat", [128, 128], FP32, isOutput=False)
    lmat = nc.declare_dram_parameter("lmat", [128, 128], BF16, isOutput=False)
    out = nc.declare_dram_parameter("out", [bpc, t, J, C], FP32, isOutput=True)

    nchunks = bpc // bchunk
    fd = bchunk * JC            # tile free dim (2048)
    hfd = fd // 2               # half free dim (1024)
    sd = bchunk * J             # dot free dim (512)
    ntiles = (t + TSTRIDE - 1) // TSTRIDE

    with tile.TileContext(nc) as tc, ExitStack() as ctx:
        consts = ctx.enter_context(tc.tile_pool(name="consts", bufs=1))
        qpool = ctx.enter_context(tc.tile_pool(name="qpool", bufs=3))
        opool = ctx.enter_context(tc.tile_pool(name="opool", bufs=3))
        spool = ctx.enter_context(tc.tile_pool(name="spool", bufs=3))
        qpvp = ctx.enter_context(tc.tile_pool(name="qpvp", bufs=3, space="PSUM"))
        prefp = ctx.enter_context(tc.tile_pool(name="prefp", bufs=2, space="PSUM"))

        smatSB = consts.tile([128, 128], FP32)
        nc.sync.dma_start(out=smatSB[:, :], in_=smat[:, :])
        lmatSB = consts.tile([128, 128], BF16)
        nc.sync.dma_start(out=lmatSB[:, :], in_=lmat[:, :])

        for rep_cc in range(reps * nchunks):
            cc = rep_cc % nchunks
            b0 = cc * bchunk
            sig_prev = None
            for i in range(ntiles):
                t0 = TSTRIDE * i
                nv = min(TSTRIDE, t - t0)
                rows = nv + 1

                qt = qpool.tile([128, fd], FP32, tag="qt")
                qt3 = qt.rearrange("p (b x) -> p b x", b=bchunk)
                if i == 0:
                    nc.sync.dma_start(
                        out=qt3[1:rows],
                        in_=q[b0:b0 + bchunk, 0:nv, :, :]
                        .rearrange("b t j c -> t b (j c)"),
                    )
                    nc.sync.dma_start(
                        out=qt3[0:1],
                        in_=q[b0:b0 + bchunk, 0:1, :, :]
                        .rearrange("b t j c -> t b (j c)"),
                    )
                else:
                    nc.sync.dma_start(
                        out=qt3[0:rows],
                        in_=q[b0:b0 + bchunk, t0 - 1:t0 + nv, :, :]
                        .rearrange("b t j c -> t b (j c)"),
                    )

                # o holds prod, later overwritten with the final output
                o = opool.tile([128, fd], FP32, tag="o")
                for h in range(2):
                    qpv = qpvp.tile([128, hfd], FP32, tag="qpv")
                    for lo in range(0, hfd, MMN):
                        w = min(MMN, hfd - lo)
                        nc.tensor.matmul(
                            qpv[:rows, lo:lo + w],
                            lhsT=smatSB[:rows, :rows],
                            rhs=qt[:rows, h * hfd + lo:h * hfd + lo + w],
                            start=True,
                            stop=True,
                        )
                    nc.vector.tensor_tensor(
                        out=o[:rows, h * hfd:(h + 1) * hfd],
                        in0=qt[:rows, h * hfd:(h + 1) * hfd],
                        in1=qpv[:rows],
                        op=Alu.mult,
                    )

                # d = pairwise dot over c:  (c0+c1) + (c2+c3)
                u = spool.tile([128, 2 * sd], FP32, tag="u")
                ov = o[:rows].rearrange("p (s k two) -> p s k two", k=2, two=2)
                uv = u[:rows].rearrange("p (s two) -> p s two", two=2)
                nc.vector.tensor_tensor(
                    out=uv, in0=ov[:, :, :, 0], in1=ov[:, :, :, 1], op=Alu.add
                )
                dt_ = spool.tile([128, sd], FP32, tag="dt")
                nc.vector.tensor_tensor(
                    out=dt_[:rows], in0=uv[:, :, 0], in1=uv[:, :, 1], op=Alu.add
                )

                sg = spool.tile([128, sd], FP32, tag="sg")
                nc.scalar.activation(sg[:rows], dt_[:rows], Act.Sign, scale=-1.0)
                e = spool.tile([128, sd], BF16, tag="e")
                nc.scalar.activation(e[:rows], sg[:rows], Act.Relu)
                if i == 0:
                    nc.scalar.mul(e[0:1], e[0:1], 0.0)
                else:
                    # carry: previous tile's sigma[last] -> parity into e[0]
                    sigscr = spool.tile([1, sd], BF16, tag="sigscr")
                    nc.sync.dma_start(out=sigscr[0:1], in_=sig_prev)
                    nc.scalar.activation(
                        e[0:1], sigscr[0:1], Act.Copy, bias=0.5, scale=-0.5
                    )

                pref = prefp.tile([128, sd], FP32, tag="pref")
                nc.tensor.matmul(
                    pref[:rows],
                    lhsT=lmatSB[:rows, :rows],
                    rhs=e[:rows],
                    start=True,
                    stop=True,
                )

                # parity -> sigma in {+1, -1}: int cast, &1, then 1 - 2*p
                prefi = spool.tile([128, sd], I32, tag="prefi")
                nc.vector.tensor_copy(out=prefi[:rows], in_=pref[:rows])
                p01 = spool.tile([128, sd], I32, tag="p01")
                nc.vector.tensor_scalar(
                    out=p01[:rows], in0=prefi[:rows], scalar1=1, scalar2=None,
                    op0=Alu.bitwise_and,
                )
                sig = spool.tile([128, sd], BF16, tag="sig")
                nc.scalar.activation(
                    sig[:rows], p01[:rows], Act.Copy, bias=1.0, scale=-2.0
                )
                sig_prev = sig[rows - 1:rows]

                # out = q * sigma (broadcast over c) -> multiply by +/-1
                nc.gpsimd.tensor_tensor(
                    out=o[:rows].rearrange("p (s c) -> p s c", c=C),
                    in0=qt[:rows].rearrange("p (s c) -> p s c", c=C),
                    in1=_bcast_c(sig[:rows]),
                    op=Alu.mult,
                )

                nc.sync.dma_start(
                    out=out[b0:b0 + bchunk, t0:t0 + nv, :, :]
                    .rearrange("b t j c -> t b (j c)"),
                    in_=o.rearrange("p (b x) -> p b x", b=bchunk)[1:1 + nv],
                )
    return nc


def make_consts():
    smat = np.eye(128, k=1, dtype=np.float32)      # S[k, m] = 1 iff m == k+1
    lmat = np.triu(np.ones((128, 128), np.float32)).astype(ml_dtypes.bfloat16)
    return smat, lmat


def kernel(joint_rotations: np.ndarray) -> np.ndarray:
    q = np.ascontiguousarray(joint_rotations, dtype=np.float32)
    assert q.shape == (B, T, J, C)
    smat, lmat = make_consts()
    nc = build_nc()
    nc.finalize()   # run bacc passes (wait splitting, reg alloc) + freeze
    in_maps = [
        {"q": q[c * BPC:(c + 1) * BPC], "smat": smat, "lmat": lmat}
        for c in range(NCORES)
    ]
    res = run_bass_kernel_spmd(nc, in_maps, list(range(NCORES)))
    outs = [np.asarray(r["out"]) for r in res.results]
    return np.concatenate(outs, axis=0)


# revision 23
# speedup vs baseline: 15.0392x; 1.2606x over previous
"""Trainium2 Bass kernel for BatchRemoveQuatDiscontinuities.

Algorithm (per (batch, joint) lane):
    d[t]    = dot(q[t], q[t-1])                (fp32, 4-wide dot)
    flip[t] = 1 if d[t] < 0 else 0             (t >= 1; flip[0] = 0)
    sigma[t] = (-1)^(sum_{s<=t} flip[s])       (cumulative sign parity)
    out[t]  = q[t] * sigma[t]

Mapping on a NeuronCore (data-parallel over batch across 8 cores):
  * Tiles put time on the partition axis: tile i holds q rows
    t in [127*i - 1, 127*i + 126] (one-row overlap), free dim = (b, j, c).
  * The one-step time shift is a TensorE matmul with an off-diagonal
    0/1 matrix S (fp32: the single 1.0*q product is exact).
  * prod = q .* q_shift on VectorE, dot via tensor_reduce over c.
  * flip indicator on ScalarE: e = Relu(Sign(-d)) in bf16.
  * The prefix sum over time (partitions) is a TensorE matmul with an
    upper-triangular ones matrix (bf16 inputs, fp32 PSUM accumulate:
    counts <= 1024 stay exact).  e[0] is 0 by construction, so the
    cross-tile carry (prefix parity, 0/1, bf16-exact) is DMA'd into
    e[0] of the next tile; the triangular matmul then adds it to every
    row's count, which is all the parity needs.
  * Parity: VectorE casts the prefix to int32 and takes (n & 1);
    ScalarE turns it into sigma = 1 - 2*(n & 1) in {+1, -1} (bf16).
    GpSimd multiplies out = q * sigma (broadcast over c) — a +/-1.0
    multiply, exact in fp32.
"""

import numpy as np
import ml_dtypes
from contextlib import ExitStack

import concourse.bass as bass
import concourse.bacc as bacc
import concourse.tile as tile
from concourse import mybir
from concourse.bass_utils import run_bass_kernel_spmd

B, T, J, C = 128, 1024, 64, 4
NCORES = 8
JC = J * C                      # 256 floats, 1KB contiguous per (b, t)
BPC = B // NCORES               # 16 batch clips per core
BCHUNK = 8                      # clips per tile (free dim = BCHUNK*JC = 2048)
TSTRIDE = 127                   # valid t rows produced per full tile
MMN = 512                       # max matmul free dim (one PSUM bank fp32)

FP32 = mybir.dt.float32
BF16 = mybir.dt.bfloat16
I32 = mybir.dt.int32
Alu = mybir.AluOpType
Act = mybir.ActivationFunctionType


def _bcast_c(ap3, c=C):
    """[P, S] AP -> [P, S, c] AP broadcasting along a new innermost dim."""
    return bass.AP(
        tensor=ap3.tensor,
        offset=ap3.offset,
        ap=[*[list(d) for d in ap3.ap], [0, c]],
    )


def build_nc(bpc=BPC, t=T, bchunk=BCHUNK, reps=1, mode="full"):
    # mode: "full" | "dma" (loads+stores only) | "noqp" (no shift/prod)
    #       | "nogp" (no final gpsimd multiply) | "nopar" (no parity chain)
    nc = bacc.Bacc(None, target_bir_lowering=False)
    q = nc.declare_dram_parameter("q", [bpc, t, J, C], FP32, isOutput=False)
    smat = nc.declare_dram_parameter("smat", [128, 128], FP32, isOutput=False)
    lmat = nc.declare_dram_parameter("lmat", [128, 128], BF16, isOutput=False)
    out = nc.declare_dram_parameter("out", [bpc, t, J, C], FP32, isOutput=True)

    nchunks = bpc // bchunk
    fd = bchunk * JC            # tile free dim (2048)
    hfd = fd // 2               # half free dim (1024)
    sd = bchunk * J             # dot free dim (512)
    ntiles = (t + TSTRIDE - 1) // TSTRIDE

    with tile.TileContext(nc) as tc, ExitStack() as ctx:
        consts = ctx.enter_context(tc.tile_pool(name="consts", bufs=1))
        qpool = ctx.enter_context(tc.tile_pool(name="qpool", bufs=3))
        opool = ctx.enter_context(tc.tile_pool(name="opool", bufs=3))
        spool = ctx.enter_context(tc.tile_pool(name="spool", bufs=3))
        qpvp = ctx.enter_context(tc.tile_pool(name="qpvp", bufs=3, space="PSUM"))
        prefp = ctx.enter_context(tc.tile_pool(name="prefp", bufs=2, space="PSUM"))

        smatSB = consts.tile([128, 128], FP32)
        nc.sync.dma_start(out=smatSB[:, :], in_=smat[:, :])
        lmatSB = consts.tile([128, 128], BF16)
        nc.sync.dma_start(out=lmatSB[:, :], in_=lmat[:, :])

        for rep_cc in range(reps * nchunks):
            cc = rep_cc % nchunks
            b0 = cc * bchunk
            sig_prev = None
            for i in range(ntiles):
                t0 = TSTRIDE * i
                nv = min(TSTRIDE, t - t0)
                rows = nv + 1

                qt = qpool.tile([128, fd], FP32, tag="qt")
                qt3 = qt.rearrange("p (b x) -> p b x", b=bchunk)
                if i == 0:
                    nc.sync.dma_start(
                        out=qt3[1:rows],
                        in_=q[b0:b0 + bchunk, 0:nv, :, :]
                        .rearrange("b t j c -> t b (j c)"),
                    )
                    nc.sync.dma_start(
                        out=qt3[0:1],
                        in_=q[b0:b0 + bchunk, 0:1, :, :]
                        .rearrange("b t j c -> t b (j c)"),
                    )
                else:
                    nc.sync.dma_start(
                        out=qt3[0:rows],
                        in_=q[b0:b0 + bchunk, t0 - 1:t0 + nv, :, :]
                        .rearrange("b t j c -> t b (j c)"),
                    )

                # o holds prod, later overwritten with the final output
                o = opool.tile([128, fd], FP32, tag="o")
                if mode == "dma":
                    nc.sync.dma_start(
                        out=out[b0:b0 + bchunk, t0:t0 + nv, :, :]
                        .rearrange("b t j c -> t b (j c)"),
                        in_=qt.rearrange("p (b x) -> p b x", b=bchunk)[1:1 + nv],
                    )
                    continue
                for h in range(2 if mode != "noqp" else 0):
                    qpv = qpvp.tile([128, hfd], FP32, tag="qpv")
                    for lo in range(0, hfd, MMN):
                        w = min(MMN, hfd - lo)
                        nc.tensor.matmul(
                            qpv[:rows, lo:lo + w],
                            lhsT=smatSB[:rows, :rows],
                            rhs=qt[:rows, h * hfd + lo:h * hfd + lo + w],
                            start=True,
                            stop=True,
                        )
                    nc.vector.tensor_tensor(
                        out=o[:rows, h * hfd:(h + 1) * hfd],
                        in0=qt[:rows, h * hfd:(h + 1) * hfd],
                        in1=qpv[:rows],
                        op=Alu.mult,
                    )

                if mode == "noqp":
                    nc.vector.tensor_copy(out=o[:rows], in_=qt[:rows])
                # d = pairwise dot over c:  (c0+c1) + (c2+c3)
                u = spool.tile([128, 2 * sd], FP32, tag="u")
                ov = o[:rows].rearrange("p (s k two) -> p s k two", k=2, two=2)
                uv = u[:rows].rearrange("p (s two) -> p s two", two=2)
                nc.vector.tensor_tensor(
                    out=uv, in0=ov[:, :, :, 0], in1=ov[:, :, :, 1], op=Alu.add
                )
                dt_ = spool.tile([128, sd], FP32, tag="dt")
                nc.vector.tensor_tensor(
                    out=dt_[:rows], in0=uv[:, :, 0], in1=uv[:, :, 1], op=Alu.add
                )

                sg = spool.tile([128, sd], FP32, tag="sg")
                nc.scalar.activation(sg[:rows], dt_[:rows], Act.Sign, scale=-1.0)
                e = spool.tile([128, sd], BF16, tag="e")
                nc.scalar.activation(e[:rows], sg[:rows], Act.Relu)
                if i == 0:
                    nc.scalar.mul(e[0:1], e[0:1], 0.0)
                else:
                    # carry: previous tile's sigma[last] -> parity into e[0]
                    sigscr = spool.tile([1, sd], BF16, tag="sigscr")
                    nc.sync.dma_start(out=sigscr[0:1], in_=sig_prev)
                    nc.scalar.activation(
                        e[0:1], sigscr[0:1], Act.Copy, bias=0.5, scale=-0.5
                    )

                sig = spool.tile([128, sd], BF16, tag="sig")
                if mode == "nopar":
                    nc.vector.tensor_scalar(
                        out=sig[:rows], in0=e[:rows], scalar1=1.0, scalar2=None,
                        op0=Alu.mult,
                    )
                else:
                    pref = prefp.tile([128, sd], FP32, tag="pref")
                    nc.tensor.matmul(
                        pref[:rows],
                        lhsT=lmatSB[:rows, :rows],
                        rhs=e[:rows],
                        start=True,
                        stop=True,
                    )

                    # parity -> sigma in {+1, -1}: int cast, &1, then 1-2p
                    prefi = spool.tile([128, sd], I32, tag="prefi")
                    nc.vector.tensor_copy(out=prefi[:rows], in_=pref[:rows])
                    p01 = spool.tile([128, sd], I32, tag="p01")
                    nc.vector.tensor_scalar(
                        out=p01[:rows], in0=prefi[:rows], scalar1=1,
                        scalar2=None, op0=Alu.bitwise_and,
                    )
                    nc.scalar.activation(
                        sig[:rows], p01[:rows], Act.Copy, bias=1.0, scale=-2.0
                    )
                sig_prev = sig[rows - 1:rows]

                # out = q * sigma (broadcast over c) -> multiply by +/-1
                if mode != "nogp":
                    nc.gpsimd.tensor_tensor(
                        out=o[:rows].rearrange("p (s c) -> p s c", c=C),
                        in0=qt[:rows].rearrange("p (s c) -> p s c", c=C),
                        in1=_bcast_c(sig[:rows]),
                        op=Alu.mult,
                    )

                nc.sync.dma_start(
                    out=out[b0:b0 + bchunk, t0:t0 + nv, :, :]
                    .rearrange("b t j c -> t b (j c)"),
                    in_=o.rearrange("p (b x) -> p b x", b=bchunk)[1:1 + nv],
                )
    return nc


def make_consts():
    smat = np.eye(128, k=1, dtype=np.float32)      # S[k, m] = 1 iff m == k+1
    lmat = np.triu(np.ones((128, 128), np.float32)).astype(ml_dtypes.bfloat16)
    return smat, lmat


def kernel(joint_rotations: np.ndarray) -> np.ndarray:
    q = np.ascontiguousarray(joint_rotations, dtype=np.float32)
    assert q.shape == (B, T, J, C)
    smat, lmat = make_consts()
    nc = build_nc()
    nc.finalize()   # run bacc passes (wait splitting, reg alloc) + freeze
    in_maps = [
        {"q": q[c * BPC:(c + 1) * BPC], "smat": smat, "lmat": lmat}
        for c in range(NCORES)
    ]
    res = run_bass_kernel_spmd(nc, in_maps, list(range(NCORES)))
    outs = [np.asarray(r["out"]) for r in res.results]
    return np.concatenate(outs, axis=0)


# revision 24
# speedup vs baseline: 87.2839x; 5.8038x over previous
"""Trainium2 Bass kernel for BatchRemoveQuatDiscontinuities.

Algorithm (per (batch, joint) lane):
    d[t]    = dot(q[t], q[t-1])                (fp32, 4-wide dot)
    flip[t] = 1 if d[t] < 0 else 0             (t >= 1; flip[0] = 0)
    sigma[t] = (-1)^(sum_{s<=t} flip[s])       (cumulative sign parity)
    out[t]  = q[t] * sigma[t]

Mapping on a NeuronCore (data-parallel over batch across 8 cores):
  * Tiles put time on the partition axis: tile i holds q rows
    t in [127*i - 1, 127*i + 126] (one-row overlap), free dim = (b, j, c).
  * The one-step time shift is a TensorE matmul with an off-diagonal
    0/1 matrix S (fp32: the single 1.0*q product is exact).
  * prod = q .* q_shift on VectorE, dot via tensor_reduce over c.
  * flip indicator on ScalarE: e = Relu(Sign(-d)) in bf16.
  * The prefix sum over time (partitions) is a TensorE matmul with an
    upper-triangular ones matrix (bf16 inputs, fp32 PSUM accumulate:
    counts <= 1024 stay exact).  e[0] is 0 by construction, so the
    cross-tile carry (prefix parity, 0/1, bf16-exact) is DMA'd into
    e[0] of the next tile; the triangular matmul then adds it to every
    row's count, which is all the parity needs.
  * Parity: VectorE casts the prefix to int32 and takes (n & 1);
    ScalarE turns it into sigma = 1 - 2*(n & 1) in {+1, -1} (bf16).
    GpSimd multiplies out = q * sigma (broadcast over c) — a +/-1.0
    multiply, exact in fp32.
"""

import numpy as np
import ml_dtypes
from contextlib import ExitStack

import concourse.bass as bass
import concourse.bacc as bacc
import concourse.tile as tile
from concourse import mybir
from concourse.bass_utils import run_bass_kernel_spmd

B, T, J, C = 128, 1024, 64, 4
NCORES = 8
JC = J * C                      # 256 floats, 1KB contiguous per (b, t)
BPC = B // NCORES               # 16 batch clips per core
BCHUNK = 8                      # clips per tile (free dim = BCHUNK*JC = 2048)
TSTRIDE = 127                   # valid t rows produced per full tile
MMN = 512                       # max matmul free dim (one PSUM bank fp32)

FP32 = mybir.dt.float32
BF16 = mybir.dt.bfloat16
I32 = mybir.dt.int32
Alu = mybir.AluOpType
Act = mybir.ActivationFunctionType


def _bcast_c(ap3, c=C):
    """[P, S] AP -> [P, S, c] AP broadcasting along a new innermost dim."""
    return bass.AP(
        tensor=ap3.tensor,
        offset=ap3.offset,
        ap=[*[list(d) for d in ap3.ap], [0, c]],
    )


def build_nc(bpc=BPC, t=T, bchunk=BCHUNK, reps=1, mode="full"):
    # mode: "full" | "dma" (loads+stores only) | "noqp" (no shift/prod)
    #       | "nogp" (no final gpsimd multiply) | "nopar" (no parity chain)
    nc = bacc.Bacc(None, target_bir_lowering=False)
    q = nc.declare_dram_parameter("q", [bpc, t, J, C], FP32, isOutput=False)
    smat = nc.declare_dram_parameter("smat", [128, 128], FP32, isOutput=False)
    lmat = nc.declare_dram_parameter("lmat", [128, 128], BF16, isOutput=False)
    out = nc.declare_dram_parameter("out", [bpc, t, J, C], FP32, isOutput=True)

    nchunks = bpc // bchunk
    fd = bchunk * JC            # tile free dim (2048)
    hfd = fd // 2               # half free dim (1024)
    sd = bchunk * J             # dot free dim (512)
    ntiles = (t + TSTRIDE - 1) // TSTRIDE

    if mode == "dmaflat":
        qf = q.rearrange("b t j c -> (b t j c)")
        of = out.rearrange("b t j c -> (b t j c)")
        nmb = (bpc * t * JC) // (128 * fd)
        with tile.TileContext(nc) as tc, ExitStack() as ctx:
            qpool = ctx.enter_context(tc.tile_pool(name="qpool", bufs=3))
            for rep in range(reps):
                for i in range(nmb):
                    qt = qpool.tile([128, fd], FP32, tag="qt")
                    sl = qf[i * 128 * fd:(i + 1) * 128 * fd]
                    nc.sync.dma_start(
                        out=qt[:, :], in_=sl.rearrange("(p x) -> p x", p=128)
                    )
                    ol = of[i * 128 * fd:(i + 1) * 128 * fd]
                    nc.sync.dma_start(
                        out=ol.rearrange("(p x) -> p x", p=128), in_=qt[:, :]
                    )
        return nc

    with tile.TileContext(nc) as tc, ExitStack() as ctx:
        consts = ctx.enter_context(tc.tile_pool(name="consts", bufs=1))
        qpool = ctx.enter_context(tc.tile_pool(name="qpool", bufs=3))
        opool = ctx.enter_context(tc.tile_pool(name="opool", bufs=3))
        spool = ctx.enter_context(tc.tile_pool(name="spool", bufs=3))
        qpvp = ctx.enter_context(tc.tile_pool(name="qpvp", bufs=3, space="PSUM"))
        prefp = ctx.enter_context(tc.tile_pool(name="prefp", bufs=2, space="PSUM"))

        smatSB = consts.tile([128, 128], FP32)
        nc.sync.dma_start(out=smatSB[:, :], in_=smat[:, :])
        lmatSB = consts.tile([128, 128], BF16)
        nc.sync.dma_start(out=lmatSB[:, :], in_=lmat[:, :])

        for rep_cc in range(reps * nchunks):
            cc = rep_cc % nchunks
            b0 = cc * bchunk
            sig_prev = None
            for i in range(ntiles):
                t0 = TSTRIDE * i
                nv = min(TSTRIDE, t - t0)
                rows = nv + 1

                qt = qpool.tile([128, fd], FP32, tag="qt")
                qt3 = qt.rearrange("p (b x) -> p b x", b=bchunk)
                if i == 0:
                    nc.sync.dma_start(
                        out=qt3[1:rows],
                        in_=q[b0:b0 + bchunk, 0:nv, :, :]
                        .rearrange("b t j c -> t b (j c)"),
                    )
                    nc.sync.dma_start(
                        out=qt3[0:1],
                        in_=q[b0:b0 + bchunk, 0:1, :, :]
                        .rearrange("b t j c -> t b (j c)"),
                    )
                else:
                    nc.sync.dma_start(
                        out=qt3[0:rows],
                        in_=q[b0:b0 + bchunk, t0 - 1:t0 + nv, :, :]
                        .rearrange("b t j c -> t b (j c)"),
                    )

                # o holds prod, later overwritten with the final output
                o = opool.tile([128, fd], FP32, tag="o")
                if mode == "dma":
                    nc.sync.dma_start(
                        out=out[b0:b0 + bchunk, t0:t0 + nv, :, :]
                        .rearrange("b t j c -> t b (j c)"),
                        in_=qt.rearrange("p (b x) -> p b x", b=bchunk)[1:1 + nv],
                    )
                    continue
                for h in range(2 if mode != "noqp" else 0):
                    qpv = qpvp.tile([128, hfd], FP32, tag="qpv")
                    for lo in range(0, hfd, MMN):
                        w = min(MMN, hfd - lo)
                        nc.tensor.matmul(
                            qpv[:rows, lo:lo + w],
                            lhsT=smatSB[:rows, :rows],
                            rhs=qt[:rows, h * hfd + lo:h * hfd + lo + w],
                            start=True,
                            stop=True,
                        )
                    nc.vector.tensor_tensor(
                        out=o[:rows, h * hfd:(h + 1) * hfd],
                        in0=qt[:rows, h * hfd:(h + 1) * hfd],
                        in1=qpv[:rows],
                        op=Alu.mult,
                    )

                if mode == "noqp":
                    nc.vector.tensor_copy(out=o[:rows], in_=qt[:rows])
                # d = pairwise dot over c:  (c0+c1) + (c2+c3)
                u = spool.tile([128, 2 * sd], FP32, tag="u")
                ov = o[:rows].rearrange("p (s k two) -> p s k two", k=2, two=2)
                uv = u[:rows].rearrange("p (s two) -> p s two", two=2)
                nc.vector.tensor_tensor(
                    out=uv, in0=ov[:, :, :, 0], in1=ov[:, :, :, 1], op=Alu.add
                )
                dt_ = spool.tile([128, sd], FP32, tag="dt")
                nc.vector.tensor_tensor(
                    out=dt_[:rows], in0=uv[:, :, 0], in1=uv[:, :, 1], op=Alu.add
                )

                sg = spool.tile([128, sd], FP32, tag="sg")
                nc.scalar.activation(sg[:rows], dt_[:rows], Act.Sign, scale=-1.0)
                e = spool.tile([128, sd], BF16, tag="e")
                nc.scalar.activation(e[:rows], sg[:rows], Act.Relu)
                if i == 0:
                    nc.scalar.mul(e[0:1], e[0:1], 0.0)
                else:
                    # carry: previous tile's sigma[last] -> parity into e[0]
                    sigscr = spool.tile([1, sd], BF16, tag="sigscr")
                    nc.sync.dma_start(out=sigscr[0:1], in_=sig_prev)
                    nc.scalar.activation(
                        e[0:1], sigscr[0:1], Act.Copy, bias=0.5, scale=-0.5
                    )

                sig = spool.tile([128, sd], BF16, tag="sig")
                if mode == "nopar":
                    nc.vector.tensor_scalar(
                        out=sig[:rows], in0=e[:rows], scalar1=1.0, scalar2=None,
                        op0=Alu.mult,
                    )
                else:
                    pref = prefp.tile([128, sd], FP32, tag="pref")
                    nc.tensor.matmul(
                        pref[:rows],
                        lhsT=lmatSB[:rows, :rows],
                        rhs=e[:rows],
                        start=True,
                        stop=True,
                    )

                    # parity -> sigma in {+1, -1}: int cast, &1, then 1-2p
                    prefi = spool.tile([128, sd], I32, tag="prefi")
                    nc.vector.tensor_copy(out=prefi[:rows], in_=pref[:rows])
                    p01 = spool.tile([128, sd], I32, tag="p01")
                    nc.vector.tensor_scalar(
                        out=p01[:rows], in0=prefi[:rows], scalar1=1,
                        scalar2=None, op0=Alu.bitwise_and,
                    )
                    nc.scalar.activation(
                        sig[:rows], p01[:rows], Act.Copy, bias=1.0, scale=-2.0
                    )
                sig_prev = sig[rows - 1:rows]

                # out = q * sigma (broadcast over c) -> multiply by +/-1
                if mode != "nogp":
                    nc.gpsimd.tensor_tensor(
                        out=o[:rows].rearrange("p (s c) -> p s c", c=C),
                        in0=qt[:rows].rearrange("p (s c) -> p s c", c=C),
                        in1=_bcast_c(sig[:rows]),
                        op=Alu.mult,
                    )

                nc.sync.dma_start(
                    out=out[b0:b0 + bchunk, t0:t0 + nv, :, :]
                    .rearrange("b t j c -> t b (j c)"),
                    in_=o.rearrange("p (b x) -> p b x", b=bchunk)[1:1 + nv],
                )
    return nc


def make_consts():
    smat = np.eye(128, k=1, dtype=np.float32)      # S[k, m] = 1 iff m == k+1
    lmat = np.triu(np.ones((128, 128), np.float32)).astype(ml_dtypes.bfloat16)
    return smat, lmat


def kernel(joint_rotations: np.ndarray) -> np.ndarray:
    q = np.ascontiguousarray(joint_rotations, dtype=np.float32)
    assert q.shape == (B, T, J, C)
    smat, lmat = make_consts()
    nc = build_nc()
    nc.finalize()   # run bacc passes (wait splitting, reg alloc) + freeze
    in_maps = [
        {"q": q[c * BPC:(c + 1) * BPC], "smat": smat, "lmat": lmat}
        for c in range(NCORES)
    ]
    res = run_bass_kernel_spmd(nc, in_maps, list(range(NCORES)))
    outs = [np.asarray(r["out"]) for r in res.results]
    return np.concatenate(outs, axis=0)
